# revision 16
# baseline (speedup 1.0000x reference)
"""Trainium2 Bass kernel for CombineAttention (B=2, T=4096, sT=1024, C=1024, H=16, D=64).

Sharding: 8 cores = 2 batches x 4 head-groups (4 heads each).
Host pre-transposes activations/weights so every on-device matmul has its
contraction dim on partitions; the monotonic mask (query i attends keys
<= 4i+3) becomes a block-causal structure handled by suffix-restricted
matmuls plus one small static (128,32) diagonal-band mask.

Precision: fp16 everywhere (full PE rate, FWL weight loads) except the
attention-weights path: exp(scores) can reach e^40, beyond fp16 range,
so exp and v are bf16 and the attn@v matmul runs in bf16. PSUM
accumulation is fp32; softmax needs no max-subtraction, and a
ones-column appended to v yields the softmax normalizer for free.
Output partials are stored fp16 and summed on host in fp32.

v2 schedule: single fine-grained instruction stream that keeps the PE
warm (HAM clock gate) and the ScalarE exp pipe full:
  - q/k/v projections chopped into ~1-3.5us quanta, interleaved between
    attention score/av tile pairs so the PE never waits on ScalarE;
  - scores for the last unit's full tiles are emitted early (deep ex
    buffer) because that unit has no projection work left to hide its
    exp latency;
  - softmax normalization broadcasts 1/l via a K=1 PE matmul instead of
    a DMA (keeps the tail chain ~2us instead of ~11us);
  - c-projection for query chunk 0 runs mid-kernel; out stores are fp16
    on the gpsimd DMA queue so they never block x-slice loads on the
    in-order sync queue.
"""

import math
from contextlib import ExitStack

import numpy as np
import ml_dtypes

import concourse.bass as bass
import concourse.tile as tile
from concourse import bacc, mybir
from concourse.bass import ts, ds

BF16 = mybir.dt.bfloat16
FP16 = mybir.dt.float16
FP32 = mybir.dt.float32

B = 2
C = 1024
T = 4096
ST = 1024
H = 16
D = 64
HO = 256          # head-group output channels per core (4 heads)
NCC = C // 128    # 8 contraction chunks
NTT = T // 128    # 32 key tiles
NKC = T // 512    # 8 key slices (projection streaming)
NQC = ST // 512   # 2 query chunks (attention)
NQT = ST // 128   # 8 query tiles (c-projection)
WARM_MMS = 8      # PE warmup burst; real q/k work continues the warming


def build_nc(masked: bool = True):
    nc = bacc.Bacc("TRN2", target_bir_lowering=False, debug=False, num_devices=8)
    xT = nc.dram_tensor("xT", [C, T], FP16, kind="ExternalInput").ap()
    sxT = nc.dram_tensor("sxT", [C, ST], FP16, kind="ExternalInput").ap()
    wq = nc.dram_tensor("wq", [C, HO], FP16, kind="ExternalInput").ap()
    wk = nc.dram_tensor("wk", [C, HO], FP16, kind="ExternalInput").ap()
    wv = nc.dram_tensor("wv", [C, HO], FP16, kind="ExternalInput").ap()
    wc = nc.dram_tensor("wc", [HO, C], FP16, kind="ExternalInput").ap()
    maskd = nc.dram_tensor("mask", [128, 32], BF16, kind="ExternalInput").ap()
    out = nc.dram_tensor("out", [ST, C], FP16, kind="ExternalOutput").ap()

    with tile.TileContext(nc) as tc, ExitStack() as ctx:
        const = ctx.enter_context(tc.tile_pool(name="const", bufs=1))
        big = ctx.enter_context(tc.tile_pool(name="big", bufs=1))
        xsl_pool = ctx.enter_context(tc.tile_pool(name="xsl", bufs=6))
        work = ctx.enter_context(tc.tile_pool(name="work", bufs=20))
        nrm = ctx.enter_context(tc.tile_pool(name="nrm", bufs=4))
        outw = ctx.enter_context(tc.tile_pool(name="outw", bufs=3))

        wq_sb = const.tile([128, NCC, HO], FP16, tag="wq")
        wk_sb = const.tile([128, NCC, HO], FP16, tag="wk")
        wv_sb = const.tile([128, NCC, HO], FP16, tag="wv")
        wc_sb = const.tile([128, 2, C], FP16, tag="wc")
        mask_sb = const.tile([128, 32], BF16, tag="mask")
        warm_sb = const.tile([128, 512], BF16, tag="warm")
        ones_sb = const.tile([1, 64], BF16, tag="ones")

        kT_sb = big.tile([128, 2, T], FP16, tag="kT")
        qsT_sb = big.tile([128, 2, ST], FP16, tag="qsT")
        v_sb = big.tile([128, NTT, 4, 65], BF16, tag="v")
        yT_sb = [
            big.tile([128, 2, 512], FP16, tag=f"yT{qc}", name=f"yT{qc}")
            for qc in range(NQC)
        ]

        nc.gpsimd.memset(warm_sb[:], 0.125)
        nc.gpsimd.memset(ones_sb[:], 1.0)

        with tc.tile_pool(name="psA", bufs=2, space="PSUM") as pp, \
             tc.tile_pool(name="psS", bufs=2, space="PSUM") as scp, \
             tc.tile_pool(name="psV", bufs=2, space="PSUM") as avp:

            # ---- PE warmup: bridge from kernel start until the first
            # x/weight slices land; real projections continue the burst ----
            wps = pp.tile([128, 512], FP32, tag="proj", name="warmps")
            for i in range(WARM_MMS):
                nc.tensor.matmul(
                    wps[:], warm_sb[:, 0:128], warm_sb[:], start=True, stop=True
                )

            # ---------------- DMA emission helpers ----------------
            def dma_pair_x(kc0, split=1):
                """Start DMAs for key slices kc0, kc0+1; returns xsl tiles.
                split>1 chops each 128KB chunk across multiple DMA queues to
                cut the per-chunk latency (~6us per 128KB on one queue)."""
                xsl = []
                for j in range(2):
                    sl = xsl_pool.tile(
                        [128, NCC, 512], FP16, tag="xsl", name=f"xsl{kc0 + j}"
                    )
                    w = 512 // split
                    for cc in range(NCC):
                        for p in range(split):
                            nc.sync.dma_start(
                                sl[:, cc, ds(p * w, w)],
                                xT[ts(cc, 128),
                                   ds((kc0 + j) * 512 + p * w, w)],
                            )
                    xsl.append(sl)
                return xsl

            # ---------------- PE work quanta ----------------
            def qp_quantum(sxsl, qc, ot):
                """q projection for (query chunk qc, channel half ot): 8 MMs."""
                ps = pp.tile([128, 512], FP32, tag="proj", name=f"pq{qc}{ot}")
                for cc in range(NCC):
                    nc.tensor.matmul(
                        ps[:],
                        wq_sb[:, cc, ts(ot, 128)],
                        sxsl[qc][:, cc, :],
                        start=(cc == 0),
                        stop=(cc == NCC - 1),
                    )
                nc.vector.tensor_copy(qsT_sb[:, ot, ts(qc, 512)], ps[:])

            def k_quantum(xsl, kc0, ot):
                """k projection for slices kc0,kc0+1 (one channel half): 16 MMs
                sharing stationary loads across the j-pair."""
                pk = [pp.tile([128, 512], FP32, tag="proj", name=f"pk{j}")
                      for j in range(2)]
                for cc in range(NCC):
                    for j in range(2):
                        nc.tensor.matmul(
                            pk[j][:],
                            wk_sb[:, cc, ts(ot, 128)],
                            xsl[j][:, cc, :],
                            start=(cc == 0),
                            stop=(cc == NCC - 1),
                        )
                for j in range(2):
                    nc.vector.tensor_copy(kT_sb[:, ot, ts(kc0 + j, 512)], pk[j][:])

            def v_quantum(xsl, kc0, j, tl):
                """v projection for one 128-key tile: 8 MMs of N=256."""
                tt = 4 * (kc0 + j) + tl
                ps = pp.tile([128, 512], FP32, tag="proj", name="pv")
                pv = ps[:, 0:256]
                for cc in range(NCC):
                    nc.tensor.matmul(
                        pv,
                        xsl[j][:, cc, ts(tl, 128)],
                        wv_sb[:, cc, :],
                        start=(cc == 0),
                        stop=(cc == NCC - 1),
                    )
                nc.vector.tensor_copy(
                    v_sb[:, tt, :, 0:64], pv.rearrange("p (h d) -> p h d", h=4)
                )
                nc.vector.memset(v_sb[:, tt, :, 64:65], 1.0)

            ex_tiles = {}
            av_tiles = {}

            def tile_geom(qc, tt):
                r = tt - 16 * qc if masked else -1  # >= 0: diagonal-band tile
                col0 = 32 * r if r >= 0 else 0
                width = 512 - col0
                base = 512 - width
                return r, col0, width, base

            def ntiles_of(qc):
                return (16 * (qc + 1)) if masked else NTT

            def S(ot, qc, tt):
                """scoresT + exp + band-mask for one 128-key tile of heads
                (2*ot, 2*ot+1), queries [512*qc, 512*qc+512)."""
                r, col0, width, base = tile_geom(qc, tt)
                # both heads' scores go into one 2-bank psum tile, h0 at the
                # end of bank 0 and h1 at the start of bank 1, so a single
                # gap-free ACTIVATE (352-cycle fixed cost) covers the pair;
                # the two heads' matmuls run concurrently on the upper/lower
                # halves of the PE array (row tiling via base_partition)
                sc = scp.tile([128, 1024], FP32, tag="sc")
                for h in range(2):
                    row = ds(64 * h, 64)
                    nc.tensor.matmul(
                        sc[:, ds(base + width * h, width)],
                        kT_sb[row, ot, ts(tt, 128)],
                        qsT_sb[row, ot, ds(512 * qc + col0, width)],
                        start=True,
                        stop=True,
                    )
                ex = work.tile([128, 1024], BF16, tag="exp", name=f"ex{ot}{qc}{tt}")
                nc.scalar.activation(
                    ex[:, ds(base, 2 * width)],
                    sc[:, ds(base, 2 * width)],
                    mybir.ActivationFunctionType.Exp,
                )
                if r >= 0:
                    exb = ex[:, ds(base, 2 * width)].rearrange(
                        "p (g x) -> p g x", g=2
                    )[:, :, 0:32]
                    nc.vector.tensor_mul(
                        exb,
                        exb,
                        mask_sb[:].unsqueeze(1).broadcast_to([128, 2, 32]),
                    )
                ex_tiles[(ot, qc, tt)] = ex

            def A(ot, qc, tt):
                """attn @ v_aug accumulation for one key tile."""
                ntiles = ntiles_of(qc)
                if tt == 0:
                    av_tiles[(ot, qc)] = [
                        avp.tile([65, 512], FP32, tag="av", name=f"av{ot}{qc}{hh}")
                        for hh in range(2)
                    ]
                avps = av_tiles[(ot, qc)]
                r, col0, width, base = tile_geom(qc, tt)
                ex = ex_tiles.pop((ot, qc, tt))
                for h in range(2):
                    nc.tensor.matmul(
                        avps[h][:, ds(col0, width)],
                        v_sb[:, tt, 2 * ot + h, :],
                        ex[:, ds(base + width * h, width)],
                        start=(tt == 0),
                        stop=(tt == ntiles - 1),
                    )

            norm_state = {}

            def norm_dve(ot, qc):
                """First half of y = yT_unnorm / l: copy av+l to SBUF and
                compute 1/l (all DVE). Emitted right after the unit's last av
                matmul so the chain runs while the PE does other work."""
                avps = av_tiles.pop((ot, qc))
                st = []
                # reciprocal chains first (they gate the bc matmuls), bulk
                # av copies after (only needed by the final multiply)
                for h in range(2):
                    lsb = nrm.tile([1, 512], FP32, tag="lsb")
                    nc.vector.tensor_copy(lsb[:], avps[h][64:65, :])
                    linv = nrm.tile([1, 512], FP32, tag="linv")
                    # custom-DVE recip needs a partition-0 SBUF input
                    nc.vector.reciprocal_approx_fast(linv[:], lsb[:])
                    linvb = nrm.tile([1, 512], BF16, tag="linvb")
                    nc.vector.tensor_copy(linvb[:], linv[:])
                    st.append([None, linvb])
                for h in range(2):
                    avsb = nrm.tile([64, 512], FP32, tag="avsb", name=f"avsb{h}")
                    nc.vector.tensor_copy(avsb[:], avps[h][0:64, :])
                    st[h][0] = avsb
                norm_state[(ot, qc)] = st

            def norm_fin(ot, qc):
                """Second half: K=1 PE matmul broadcasts 1/l across 64
                partitions, one DVE multiply writes normalized yT (fp16).
                Must precede the next unit's first av matmul (psum reuse)."""
                st = norm_state.pop((ot, qc))
                bcs = []
                for h, (avsb, linvb) in enumerate(st):
                    bc = avp.tile([64, 512], FP32, tag="av", name=f"bc{h}")
                    nc.tensor.matmul(
                        bc[:], ones_sb[:], linvb[:], start=True, stop=True
                    )
                    bcs.append(bc)
                for h, (avsb, linvb) in enumerate(st):
                    nc.vector.tensor_mul(
                        yT_sb[qc][ds(64 * h, 64), ot, :], avsb[:], bcs[h][:]
                    )

            def cp(nt):
                """c-projection + fp16 store for one 128-query tile."""
                po = [pp.tile([128, 512], FP32, tag="proj", name=f"po{ec}")
                      for ec in range(2)]
                for kk in range(2):
                    for ec in range(2):
                        nc.tensor.matmul(
                            po[ec][:],
                            yT_sb[nt // 4][:, kk, ts(nt % 4, 128)],
                            wc_sb[:, kk, ts(ec, 512)],
                            start=(kk == 0),
                            stop=(kk == 1),
                        )
                for ec in range(2):
                    osb = outw.tile([128, 512], FP16, tag="osb")
                    nc.vector.tensor_copy(osb[:], po[ec][:])
                    # gpsimd DMA queue: out stores must not block x-slice
                    # loads on the in-order sync queue
                    nc.gpsimd.dma_start(out[ts(nt, 128), ts(ec, 512)], osb[:])

            def cp_kk0(nt, po):
                """first-half (kk=0) c-proj matmuls into caller-provided
                psum APs; pre-runnable once unit (0, qc) is normalized."""
                for ec in range(2):
                    nc.tensor.matmul(
                        po[ec],
                        yT_sb[nt // 4][:, 0, ts(nt % 4, 128)],
                        wc_sb[:, 0, ts(ec, 512)],
                        start=True,
                        stop=False,
                    )

            def cp_kk1(nt, po):
                """second-half (kk=1) c-proj matmuls + fp16 store."""
                for ec in range(2):
                    nc.tensor.matmul(
                        po[ec],
                        yT_sb[nt // 4][:, 1, ts(nt % 4, 128)],
                        wc_sb[:, 1, ts(ec, 512)],
                        start=False,
                        stop=True,
                    )
                for ec in range(2):
                    osb = outw.tile([128, 512], FP16, tag="osb")
                    nc.vector.tensor_copy(osb[:], po[ec])
                    nc.gpsimd.dma_start(out[ts(nt, 128), ts(ec, 512)], osb[:])

            # ---------------- masked (monotonic) schedule ----------------
            if masked:
                # sxsl tiles for q projection
                sxsl = [
                    xsl_pool.tile([128, NCC, 512], FP16, tag="xsl", name=f"sxsl{qc}")
                    for qc in range(NQC)
                ]
                # DMA priority order: wk + pair0 feed the long k/v pole;
                # wq + sx feed q projection; later pairs stream behind.
                # Early chunks are split across queues for latency.
                for cc in range(NCC):
                    for p in range(2):
                        nc.sync.dma_start(
                            wk_sb[:, cc, ts(p, 128)],
                            wk[ts(cc, 128), ts(p, 128)])
                xp0 = dma_pair_x(0, split=4)
                for cc in range(NCC):
                    for p in range(2):
                        nc.sync.dma_start(
                            wq_sb[:, cc, ts(p, 128)],
                            wq[ts(cc, 128), ts(p, 128)])
                        nc.sync.dma_start(
                            sxsl[0][:, cc, ts(p, 256)],
                            sxT[ts(cc, 128), ds(p * 256, 256)])
                for cc in range(NCC):
                    for p in range(2):
                        nc.sync.dma_start(
                            sxsl[1][:, cc, ts(p, 256)],
                            sxT[ts(cc, 128), ds(512 + p * 256, 256)])
                for cc in range(NCC):
                    nc.sync.dma_start(wv_sb[:, cc, :], wv[ts(cc, 128), :])
                nc.sync.dma_start(mask_sb[:], maskd[:])
                xp2 = dma_pair_x(2, split=2)
                for kk in range(2):
                    nc.sync.dma_start(wc_sb[:, kk, :], wc[ts(kk, 128), :])
                # pair4 reuses sxsl0's buffer (waits on q-proj qc0), pair6
                # reuses pair0's (waits on k/v of pair0) — emitted now, the
                # semaphores resolve the timing.
                xp4 = dma_pair_x(4)
                xp6 = dma_pair_x(6)

                # Filler queue: projection/cproj quanta consumed between
                # attention tile pairs. Order respects data availability.
                filler = []
                filler.append(lambda: k_quantum(xp0, 0, 0))
                filler.append(lambda: k_quantum(xp0, 0, 1))
                filler.append(lambda: qp_quantum(sxsl, 0, 0))
                filler.append(lambda: qp_quantum(sxsl, 0, 1))
                for j in range(2):
                    for tl in range(4):
                        filler.append(
                            lambda j=j, tl=tl: v_quantum(xp0, 0, j, tl))
                filler.append(lambda: qp_quantum(sxsl, 1, 0))
                filler.append(lambda: qp_quantum(sxsl, 1, 1))
                filler.append(lambda: k_quantum(xp2, 2, 0))
                filler.append(lambda: k_quantum(xp2, 2, 1))
                for j in range(2):
                    for tl in range(4):
                        filler.append(
                            lambda j=j, tl=tl: v_quantum(xp2, 2, j, tl))
                filler.append(lambda: k_quantum(xp4, 4, 0))
                filler.append(lambda: k_quantum(xp4, 4, 1))
                for j in range(2):
                    for tl in range(4):
                        filler.append(
                            lambda j=j, tl=tl: v_quantum(xp4, 4, j, tl))
                filler.append(lambda: k_quantum(xp6, 6, 0))
                filler.append(lambda: k_quantum(xp6, 6, 1))
                for j in range(2):
                    for tl in range(4):
                        filler.append(
                            lambda j=j, tl=tl: v_quantum(xp6, 6, j, tl))
                for nt in range(4):
                    filler.append(lambda nt=nt: cp(nt))

                fill_pos = 0

                def pull(n):
                    nonlocal fill_pos
                    for _ in range(n):
                        if fill_pos < len(filler):
                            filler[fill_pos]()
                            fill_pos += 1

                def pull_through(idx):
                    nonlocal fill_pos
                    while fill_pos <= idx:
                        filler[fill_pos]()
                        fill_pos += 1

                # filler indices: 0-1 k0, 2-3 qp(qc0), 4-11 v(pair0),
                # 12-13 qp(qc1), 14-15 k2, 16-23 v2, 24-25 k4, 26-33 v4,
                # 34-35 k6, 36-43 v6, 44-45 cp(0..1)
                IDX_QP0 = 3
                IDX_K2 = 15
                IDX_K4 = 25
                IDX_K6 = 35

                # --- Unit 0 = (ot 0, qc 0): 16 diagonal tiles ---
                pull_through(IDX_QP0)          # k0 + q(qc0)
                for t in range(0, 8):
                    S(0, 0, t)
                    if t >= 2:
                        A(0, 0, t - 2)
                    pull(1)                    # v(pair0), q(qc1) stream in
                pull_through(IDX_K2)
                for t in range(8, 16):
                    S(0, 0, t)
                    A(0, 0, t - 2)
                    pull(1)
                A(0, 0, 14)
                A(0, 0, 15)
                norm_dve(0, 0)
                # --- Unit 1 = (1, 0): next unit's scores + filler cover the
                # norm chain and av-psum handover ---
                S(1, 0, 0)
                S(1, 0, 1)
                norm_fin(0, 0)
                pull(2)                        # k4 quanta
                for t in range(2, 16):
                    S(1, 0, t)
                    A(1, 0, t - 2)
                    if t in (4, 8, 12):
                        pull(1)
                A(1, 0, 14)
                A(1, 0, 15)
                norm_dve(1, 0)
                pull_through(IDX_K4)
                # --- Unit 2 = (0, 1): 32 tiles; also pre-emit the first 12 of
                # unit 3's full-tile scores so its exp runs ahead on ScalarE.
                # Filler is paced to last through the whole unit (ScalarE is
                # the laggard here; the PE needs projection work to chew on) ---
                S(0, 1, 0)
                S(0, 1, 1)
                pull(1)
                norm_fin(1, 0)
                u3_t = 0
                for t in range(2, 32):
                    S(0, 1, t)
                    A(0, 1, t - 2)
                    if t % 2 == 0 or t > 25:
                        pull(1)
                    if t % 2 == 1 and u3_t < 12:
                        S(1, 1, u3_t)
                        u3_t += 1
                    if t == 23:
                        pull_through(IDX_K6)
                A(0, 1, 30)
                A(0, 1, 31)
                norm_dve(0, 1)
                # --- Unit 3 = (1, 1): finish its full-tile scores (covers
                # unit 2's norm chain), then interleave full-tile av with
                # diagonal scores+av so the PE tracks ScalarE's exp progress.
                # c-proj kk0 matmuls for nt 4..6 pre-run here (they only need
                # unit 2's normalized half); kk1 + stores follow the last
                # norm, leaving a ~2us tail.
                pull(99)
                norm_fin(0, 1)
                while u3_t < 16:
                    S(1, 1, u3_t)
                    u3_t += 1
                S(1, 1, 16)
                S(1, 1, 17)
                po_pre = {}
                for i in range(14):
                    A(1, 1, i)            # full tile (start group at i == 0)
                    if i >= 3:
                        A(1, 1, 13 + i)   # diagonal tiles 16..26
                    if 18 + i < 32:
                        S(1, 1, 18 + i)
                # last scores are out; scp pool is safe to borrow for c-proj
                po_pre[4] = [pp.tile([128, 512], FP32, tag="proj",
                                     name=f"po4{ec}")[:] for ec in range(2)]
                cp_kk0(4, po_pre[4])
                A(1, 1, 14)
                A(1, 1, 27)
                sc5 = scp.tile([128, 1024], FP32, tag="sc", name="po5")
                po_pre[5] = [sc5[:, ts(ec, 512)] for ec in range(2)]
                cp_kk0(5, po_pre[5])
                A(1, 1, 15)
                A(1, 1, 28)
                sc6 = scp.tile([128, 1024], FP32, tag="sc", name="po6")
                po_pre[6] = [sc6[:, ts(ec, 512)] for ec in range(2)]
                cp_kk0(6, po_pre[6])
                for t in (29, 30, 31):
                    A(1, 1, t)            # stop group fires on tile 31
                norm_dve(1, 1)
                norm_fin(1, 1)
                for nt in range(4, 7):
                    cp_kk1(nt, po_pre[nt])
                cp(7)
            else:
                # unmasked fallback: coarse sequential schedule
                sxsl = [
                    xsl_pool.tile([128, NCC, 512], FP16, tag="xsl", name=f"sxsl{qc}")
                    for qc in range(NQC)
                ]
                for cc in range(NCC):
                    nc.sync.dma_start(wq_sb[:, cc, :], wq[ts(cc, 128), :])
                    nc.sync.dma_start(sxsl[0][:, cc, :], sxT[ts(cc, 128), ts(0, 512)])
                for cc in range(NCC):
                    nc.sync.dma_start(wk_sb[:, cc, :], wk[ts(cc, 128), :])
                    nc.sync.dma_start(sxsl[1][:, cc, :], sxT[ts(cc, 128), ts(1, 512)])
                nc.sync.dma_start(wv_sb[:], wv.rearrange("(cc p) o -> p cc o", p=128))
                nc.sync.dma_start(mask_sb[:], maskd[:])
                for kk in range(2):
                    nc.sync.dma_start(wc_sb[:, kk, :], wc[ts(kk, 128), :])
                for qc in range(NQC):
                    for ot in range(2):
                        qp_quantum(sxsl, qc, ot)
                for kc in range(0, NKC, 2):
                    xp = dma_pair_x(kc)
                    for ot in range(2):
                        k_quantum(xp, kc, ot)
                    for j in range(2):
                        for tl in range(4):
                            v_quantum(xp, kc, j, tl)
                for qc in range(NQC):
                    for ot in range(2):
                        for t in range(NTT):
                            S(ot, qc, t)
                            if t >= 2:
                                A(ot, qc, t - 2)
                        A(ot, qc, NTT - 2)
                        A(ot, qc, NTT - 1)
                        norm_dve(ot, qc)
                        norm_fin(ot, qc)
                for nt in range(NQT):
                    cp(nt)

    nc.compile()
    return nc


_NC_CACHE = {}


def _get_nc(masked: bool):
    if masked not in _NC_CACHE:
        _NC_CACHE[masked] = build_nc(masked)
    return _NC_CACHE[masked]


def _shard_inputs(x, sx, Wq, Wk, Wv, Wc, qm):
    f16 = np.float16
    bf = ml_dtypes.bfloat16
    t_len = x.shape[1]
    qscale = math.log(t_len) / math.sqrt(D)
    qmfull = np.tile(np.asarray(qm, np.float32), 4) * qscale  # (256,)

    tk = np.arange(128)[:, None]
    cl = np.arange(32)[None, :]
    mask = (cl >= tk // 4).astype(np.float32).astype(bf)

    in_maps = []
    for b in range(B):
        xT = np.ascontiguousarray(x[b].T).astype(f16)
        sxT = np.ascontiguousarray(sx[b].T).astype(f16)
        for hg in range(4):
            sl = slice(hg * HO, (hg + 1) * HO)
            in_maps.append(
                {
                    "xT": xT,
                    "sxT": sxT,
                    "wq": np.ascontiguousarray(
                        (Wq[sl, :] * qmfull[:, None]).T
                    ).astype(f16),
                    "wk": np.ascontiguousarray(Wk[sl, :].T).astype(f16),
                    "wv": np.ascontiguousarray(Wv[sl, :].T).astype(f16),
                    "wc": np.ascontiguousarray(Wc[:, sl].T).astype(f16),
                    "mask": mask,
                }
            )
    return in_maps


def _run(inputs, trace=False):
    from concourse.bass_utils import run_bass_kernel_spmd

    x = np.asarray(inputs["x"], np.float32)
    sx = np.asarray(inputs["sx"], np.float32)
    Wq = np.asarray(inputs["Wq"], np.float32)
    Wk = np.asarray(inputs["Wk"], np.float32)
    Wv = np.asarray(inputs["Wv"], np.float32)
    Wc = np.asarray(inputs["Wc"], np.float32)
    qm = np.asarray(inputs["qm"], np.float32)
    causal = int(np.asarray(inputs.get("causal", 1)))
    masked = bool(causal) and sx.shape[1] != x.shape[1]

    nc = _get_nc(masked)
    in_maps = _shard_inputs(x, sx, Wq, Wk, Wv, Wc, qm)
    kwargs = {}
    if trace:
        kwargs = dict(trace=True, trace_cores=list(range(8)))
    res = run_bass_kernel_spmd(nc, in_maps, core_ids=list(range(8)), **kwargs)

    out = np.zeros((B, ST, C), np.float32)
    for b in range(B):
        for hg in range(4):
            out[b] += np.asarray(res.results[b * 4 + hg]["out"], np.float32)
    return out, res


def kernel(**inputs):
    out, _ = _run(inputs, trace=False)
    return out


def kernel_traced(**inputs):
    out, res = _run(inputs, trace=True)
    return out, res


# revision 18
# speedup vs baseline: 1.2911x; 1.2911x over previous
"""Trainium2 Bass kernel for CombineAttention (B=2, T=4096, sT=1024, C=1024, H=16, D=64).

Sharding: 8 cores = 2 batches x 4 head-groups (4 heads each).
Host pre-transposes activations/weights so every on-device matmul has its
contraction dim on partitions; the monotonic mask (query i attends keys
<= 4i+3) becomes a block-causal structure handled by suffix-restricted
matmuls plus one small static (128,32) diagonal-band mask.

Precision: fp16 everywhere (full PE rate, FWL weight loads) except the
attention-weights path: exp(scores) can reach e^40, beyond fp16 range,
so exp and v are bf16 and the attn@v matmul runs in bf16. PSUM
accumulation is fp32; softmax needs no max-subtraction, and a
ones-column appended to v yields the softmax normalizer for free.
Output partials are stored fp16 and summed on host in fp32.

v2 schedule: single fine-grained instruction stream that keeps the PE
warm (HAM clock gate) and the ScalarE exp pipe full:
  - q/k/v projections chopped into ~1-3.5us quanta, interleaved between
    attention score/av tile pairs so the PE never waits on ScalarE;
  - scores for the last unit's full tiles are emitted early (deep ex
    buffer) because that unit has no projection work left to hide its
    exp latency;
  - softmax normalization broadcasts 1/l via a K=1 PE matmul instead of
    a DMA (keeps the tail chain ~2us instead of ~11us);
  - c-projection for query chunk 0 runs mid-kernel; out stores are fp16
    on the gpsimd DMA queue so they never block x-slice loads on the
    in-order sync queue.
"""

import math
from contextlib import ExitStack

import numpy as np
import ml_dtypes

import concourse.bass as bass
import concourse.tile as tile
from concourse import bacc, mybir
from concourse.bass import ts, ds

BF16 = mybir.dt.bfloat16
FP16 = mybir.dt.float16
FP32 = mybir.dt.float32

B = 2
C = 1024
T = 4096
ST = 1024
H = 16
D = 64
HO = 256          # head-group output channels per core (4 heads)
NCC = C // 128    # 8 contraction chunks
NTT = T // 128    # 32 key tiles
NKC = T // 512    # 8 key slices (projection streaming)
NQC = ST // 512   # 2 query chunks (attention)
NQT = ST // 128   # 8 query tiles (c-projection)
WARM_MMS = 8      # PE warmup burst; real q/k work continues the warming


def build_nc(masked: bool = True):
    nc = bacc.Bacc("TRN2", target_bir_lowering=False, debug=False, num_devices=8)
    xT = nc.dram_tensor("xT", [C, T], FP16, kind="ExternalInput").ap()
    sxT = nc.dram_tensor("sxT", [C, ST], FP16, kind="ExternalInput").ap()
    wq = nc.dram_tensor("wq", [C, HO], FP16, kind="ExternalInput").ap()
    wk = nc.dram_tensor("wk", [C, HO], FP16, kind="ExternalInput").ap()
    wv = nc.dram_tensor("wv", [C, HO], FP16, kind="ExternalInput").ap()
    wc = nc.dram_tensor("wc", [HO, C], FP16, kind="ExternalInput").ap()
    maskd = nc.dram_tensor("mask", [128, 32], BF16, kind="ExternalInput").ap()
    out = nc.dram_tensor("out", [ST, C], FP16, kind="ExternalOutput").ap()

    with tile.TileContext(nc) as tc, ExitStack() as ctx:
        const = ctx.enter_context(tc.tile_pool(name="const", bufs=1))
        big = ctx.enter_context(tc.tile_pool(name="big", bufs=1))
        xsl_pool = ctx.enter_context(tc.tile_pool(name="xsl", bufs=6))
        work = ctx.enter_context(tc.tile_pool(name="work", bufs=20))
        nrm = ctx.enter_context(tc.tile_pool(name="nrm", bufs=4))
        outw = ctx.enter_context(tc.tile_pool(name="outw", bufs=3))

        wq_sb = const.tile([128, NCC, HO], FP16, tag="wq")
        wk_sb = const.tile([128, NCC, HO], FP16, tag="wk")
        wv_sb = const.tile([128, NCC, HO], FP16, tag="wv")
        wc_sb = const.tile([128, 2, C], FP16, tag="wc")
        mask_sb = const.tile([128, 32], BF16, tag="mask")
        warm_sb = const.tile([128, 512], BF16, tag="warm")
        ones_sb = const.tile([1, 64], BF16, tag="ones")

        kT_sb = big.tile([128, 2, T], FP16, tag="kT")
        qsT_sb = big.tile([128, 2, ST], FP16, tag="qsT")
        v_sb = big.tile([128, NTT, 4, 65], BF16, tag="v")
        yT_sb = [
            big.tile([128, 2, 512], FP16, tag=f"yT{qc}", name=f"yT{qc}")
            for qc in range(NQC)
        ]

        nc.gpsimd.memset(warm_sb[:], 0.125)
        nc.gpsimd.memset(ones_sb[:], 1.0)

        with tc.tile_pool(name="psA", bufs=2, space="PSUM") as pp, \
             tc.tile_pool(name="psS", bufs=2, space="PSUM") as scp, \
             tc.tile_pool(name="psV", bufs=2, space="PSUM") as avp:

            # ---- PE warmup: bridge from kernel start until the first
            # x/weight slices land; real projections continue the burst ----
            wps = pp.tile([128, 512], FP32, tag="proj", name="warmps")
            for i in range(WARM_MMS):
                nc.tensor.matmul(
                    wps[:], warm_sb[:, 0:128], warm_sb[:], start=True, stop=True
                )

            # ---------------- DMA emission helpers ----------------
            # Each dma_start costs ~650ns of ISSUE time on its engine's
            # sequencer (serial, in-order). Descriptors are kept big and
            # spread across the sync/scalar/gpsimd queues so issue runs in
            # parallel; per-descriptor transfer is ~22GB/s on one HW queue.
            def dma_pair_x(kc0, fine=False):
                """Start DMAs for key slices kc0, kc0+1; returns xsl tiles.
                fine=True: per-cc 128KB descriptors (lower first-chunk
                latency); else two-cc 256KB descriptors (half issue cost)."""
                xsl = []
                for j in range(2):
                    sl = xsl_pool.tile(
                        [128, NCC, 512], FP16, tag="xsl", name=f"xsl{kc0 + j}"
                    )
                    if fine:
                        for cc in range(NCC):
                            nc.sync.dma_start(
                                sl[:, cc, :], xT[ts(cc, 128), ts(kc0 + j, 512)]
                            )
                    else:
                        for cc0 in range(0, NCC, 2):
                            nc.sync.dma_start(
                                sl[:, cc0:cc0 + 2, :],
                                xT[ds(cc0 * 128, 256),
                                   ts(kc0 + j, 512)].rearrange(
                                    "(c p) w -> p c w", p=128),
                            )
                    xsl.append(sl)
                return xsl

            # ---------------- PE work quanta ----------------
            def qp_quantum(sxsl, qc, ot):
                """q projection for (query chunk qc, channel half ot): 8 MMs."""
                ps = pp.tile([128, 512], FP32, tag="proj", name=f"pq{qc}{ot}")
                for cc in range(NCC):
                    nc.tensor.matmul(
                        ps[:],
                        wq_sb[:, cc, ts(ot, 128)],
                        sxsl[qc][:, cc, :],
                        start=(cc == 0),
                        stop=(cc == NCC - 1),
                    )
                nc.vector.tensor_copy(qsT_sb[:, ot, ts(qc, 512)], ps[:])

            def k_quantum(xsl, kc0, ot):
                """k projection for slices kc0,kc0+1 (one channel half): 16 MMs
                sharing stationary loads across the j-pair."""
                pk = [pp.tile([128, 512], FP32, tag="proj", name=f"pk{j}")
                      for j in range(2)]
                for cc in range(NCC):
                    for j in range(2):
                        nc.tensor.matmul(
                            pk[j][:],
                            wk_sb[:, cc, ts(ot, 128)],
                            xsl[j][:, cc, :],
                            start=(cc == 0),
                            stop=(cc == NCC - 1),
                        )
                for j in range(2):
                    nc.vector.tensor_copy(kT_sb[:, ot, ts(kc0 + j, 512)], pk[j][:])

            def v_quantum(xsl, kc0, j, tl):
                """v projection for one 128-key tile: 8 MMs of N=256."""
                tt = 4 * (kc0 + j) + tl
                ps = pp.tile([128, 512], FP32, tag="proj", name="pv")
                pv = ps[:, 0:256]
                for cc in range(NCC):
                    nc.tensor.matmul(
                        pv,
                        xsl[j][:, cc, ts(tl, 128)],
                        wv_sb[:, cc, :],
                        start=(cc == 0),
                        stop=(cc == NCC - 1),
                    )
                nc.vector.tensor_copy(
                    v_sb[:, tt, :, 0:64], pv.rearrange("p (h d) -> p h d", h=4)
                )
                nc.vector.memset(v_sb[:, tt, :, 64:65], 1.0)

            ex_tiles = {}
            av_tiles = {}

            def tile_geom(qc, tt):
                r = tt - 16 * qc if masked else -1  # >= 0: diagonal-band tile
                col0 = 32 * r if r >= 0 else 0
                width = 512 - col0
                base = 512 - width
                return r, col0, width, base

            def ntiles_of(qc):
                return (16 * (qc + 1)) if masked else NTT

            def S(ot, qc, tt):
                """scoresT + exp + band-mask for one 128-key tile of heads
                (2*ot, 2*ot+1), queries [512*qc, 512*qc+512)."""
                r, col0, width, base = tile_geom(qc, tt)
                # both heads' scores go into one 2-bank psum tile, h0 at the
                # end of bank 0 and h1 at the start of bank 1, so a single
                # gap-free ACTIVATE (352-cycle fixed cost) covers the pair;
                # the two heads' matmuls run concurrently on the upper/lower
                # halves of the PE array (row tiling via base_partition)
                sc = scp.tile([128, 1024], FP32, tag="sc")
                for h in range(2):
                    row = ds(64 * h, 64)
                    nc.tensor.matmul(
                        sc[:, ds(base + width * h, width)],
                        kT_sb[row, ot, ts(tt, 128)],
                        qsT_sb[row, ot, ds(512 * qc + col0, width)],
                        start=True,
                        stop=True,
                    )
                ex = work.tile([128, 1024], BF16, tag="exp", name=f"ex{ot}{qc}{tt}")
                nc.scalar.activation(
                    ex[:, ds(base, 2 * width)],
                    sc[:, ds(base, 2 * width)],
                    mybir.ActivationFunctionType.Exp,
                )
                if r >= 0:
                    exb = ex[:, ds(base, 2 * width)].rearrange(
                        "p (g x) -> p g x", g=2
                    )[:, :, 0:32]
                    nc.vector.tensor_mul(
                        exb,
                        exb,
                        mask_sb[:].unsqueeze(1).broadcast_to([128, 2, 32]),
                    )
                ex_tiles[(ot, qc, tt)] = ex

            def A(ot, qc, tt):
                """attn @ v_aug accumulation for one key tile."""
                ntiles = ntiles_of(qc)
                if tt == 0:
                    av_tiles[(ot, qc)] = [
                        avp.tile([65, 512], FP32, tag="av", name=f"av{ot}{qc}{hh}")
                        for hh in range(2)
                    ]
                avps = av_tiles[(ot, qc)]
                r, col0, width, base = tile_geom(qc, tt)
                ex = ex_tiles.pop((ot, qc, tt))
                for h in range(2):
                    nc.tensor.matmul(
                        avps[h][:, ds(col0, width)],
                        v_sb[:, tt, 2 * ot + h, :],
                        ex[:, ds(base + width * h, width)],
                        start=(tt == 0),
                        stop=(tt == ntiles - 1),
                    )

            norm_state = {}

            def norm_dve(ot, qc):
                """First half of y = yT_unnorm / l: copy av+l to SBUF and
                compute 1/l (all DVE). Emitted right after the unit's last av
                matmul so the chain runs while the PE does other work."""
                avps = av_tiles.pop((ot, qc))
                st = []
                # reciprocal chains first (they gate the bc matmuls), bulk
                # av copies after (only needed by the final multiply)
                for h in range(2):
                    lsb = nrm.tile([1, 512], FP32, tag="lsb")
                    nc.vector.tensor_copy(lsb[:], avps[h][64:65, :])
                    linv = nrm.tile([1, 512], FP32, tag="linv")
                    # custom-DVE recip needs a partition-0 SBUF input
                    nc.vector.reciprocal_approx_fast(linv[:], lsb[:])
                    linvb = nrm.tile([1, 512], BF16, tag="linvb")
                    nc.vector.tensor_copy(linvb[:], linv[:])
                    st.append([None, linvb])
                for h in range(2):
                    avsb = nrm.tile([64, 512], FP32, tag="avsb", name=f"avsb{h}")
                    nc.vector.tensor_copy(avsb[:], avps[h][0:64, :])
                    st[h][0] = avsb
                norm_state[(ot, qc)] = st

            def norm_fin(ot, qc):
                """Second half: K=1 PE matmul broadcasts 1/l across 64
                partitions, one DVE multiply writes normalized yT (fp16).
                Must precede the next unit's first av matmul (psum reuse)."""
                st = norm_state.pop((ot, qc))
                bcs = []
                for h, (avsb, linvb) in enumerate(st):
                    bc = avp.tile([64, 512], FP32, tag="av", name=f"bc{h}")
                    nc.tensor.matmul(
                        bc[:], ones_sb[:], linvb[:], start=True, stop=True
                    )
                    bcs.append(bc)
                for h, (avsb, linvb) in enumerate(st):
                    nc.vector.tensor_mul(
                        yT_sb[qc][ds(64 * h, 64), ot, :], avsb[:], bcs[h][:]
                    )

            def cp(nt):
                """c-projection + fp16 store for one 128-query tile."""
                po = [pp.tile([128, 512], FP32, tag="proj", name=f"po{ec}")
                      for ec in range(2)]
                for kk in range(2):
                    for ec in range(2):
                        nc.tensor.matmul(
                            po[ec][:],
                            yT_sb[nt // 4][:, kk, ts(nt % 4, 128)],
                            wc_sb[:, kk, ts(ec, 512)],
                            start=(kk == 0),
                            stop=(kk == 1),
                        )
                for ec in range(2):
                    osb = outw.tile([128, 512], FP16, tag="osb")
                    nc.vector.tensor_copy(osb[:], po[ec][:])
                    # gpsimd DMA queue: out stores must not block x-slice
                    # loads on the in-order sync queue
                    nc.gpsimd.dma_start(out[ts(nt, 128), ts(ec, 512)], osb[:])

            def cp_kk0(nt, po):
                """first-half (kk=0) c-proj matmuls into caller-provided
                psum APs; pre-runnable once unit (0, qc) is normalized."""
                for ec in range(2):
                    nc.tensor.matmul(
                        po[ec],
                        yT_sb[nt // 4][:, 0, ts(nt % 4, 128)],
                        wc_sb[:, 0, ts(ec, 512)],
                        start=True,
                        stop=False,
                    )

            def cp_kk1(nt, po):
                """second-half (kk=1) c-proj matmuls + fp16 store."""
                for ec in range(2):
                    nc.tensor.matmul(
                        po[ec],
                        yT_sb[nt // 4][:, 1, ts(nt % 4, 128)],
                        wc_sb[:, 1, ts(ec, 512)],
                        start=False,
                        stop=True,
                    )
                for ec in range(2):
                    osb = outw.tile([128, 512], FP16, tag="osb")
                    nc.vector.tensor_copy(osb[:], po[ec])
                    nc.gpsimd.dma_start(out[ts(nt, 128), ts(ec, 512)], osb[:])

            # ---------------- masked (monotonic) schedule ----------------
            if masked:
                # sxsl tiles for q projection
                sxsl = [
                    xsl_pool.tile([128, NCC, 512], FP16, tag="xsl", name=f"sxsl{qc}")
                    for qc in range(NQC)
                ]
                # Issue plan: sync queue carries the x pairs (the long pole);
                # gpsimd carries wk/wv/mask (feeds the first k/v quanta);
                # scalar carries wq/sx/wc (feeds q projection). All three
                # issue in parallel from kernel start.
                xp0 = dma_pair_x(0, fine=True)
                for cc in range(NCC):
                    nc.gpsimd.dma_start(wk_sb[:, cc, :], wk[ts(cc, 128), :])
                for cc0 in range(0, NCC, 2):
                    nc.scalar.dma_start(
                        wq_sb[:, cc0:cc0 + 2, :],
                        wq[ds(cc0 * 128, 256), :].rearrange(
                            "(c p) o -> p c o", p=128))
                    nc.scalar.dma_start(
                        sxsl[0][:, cc0:cc0 + 2, :],
                        sxT[ds(cc0 * 128, 256), ts(0, 512)].rearrange(
                            "(c p) w -> p c w", p=128))
                for cc0 in range(0, NCC, 2):
                    nc.gpsimd.dma_start(
                        wv_sb[:, cc0:cc0 + 2, :],
                        wv[ds(cc0 * 128, 256), :].rearrange(
                            "(c p) o -> p c o", p=128))
                nc.gpsimd.dma_start(mask_sb[:], maskd[:])
                for cc0 in range(0, NCC, 2):
                    nc.scalar.dma_start(
                        sxsl[1][:, cc0:cc0 + 2, :],
                        sxT[ds(cc0 * 128, 256), ts(1, 512)].rearrange(
                            "(c p) w -> p c w", p=128))
                for kk in range(2):
                    nc.scalar.dma_start(wc_sb[:, kk, :], wc[ts(kk, 128), :])
                xp2 = dma_pair_x(2)
                # pair4 reuses sxsl0's buffer (waits on q-proj qc0), pair6
                # reuses pair0's (waits on k/v of pair0) — emitted now, the
                # semaphores resolve the timing.
                xp4 = dma_pair_x(4)
                xp6 = dma_pair_x(6)

                # Filler queue: projection/cproj quanta consumed between
                # attention tile pairs. Order respects data availability.
                filler = []
                filler.append(lambda: k_quantum(xp0, 0, 0))
                filler.append(lambda: k_quantum(xp0, 0, 1))
                filler.append(lambda: qp_quantum(sxsl, 0, 0))
                filler.append(lambda: qp_quantum(sxsl, 0, 1))
                for j in range(2):
                    for tl in range(4):
                        filler.append(
                            lambda j=j, tl=tl: v_quantum(xp0, 0, j, tl))
                filler.append(lambda: qp_quantum(sxsl, 1, 0))
                filler.append(lambda: qp_quantum(sxsl, 1, 1))
                filler.append(lambda: k_quantum(xp2, 2, 0))
                filler.append(lambda: k_quantum(xp2, 2, 1))
                for j in range(2):
                    for tl in range(4):
                        filler.append(
                            lambda j=j, tl=tl: v_quantum(xp2, 2, j, tl))
                filler.append(lambda: k_quantum(xp4, 4, 0))
                filler.append(lambda: k_quantum(xp4, 4, 1))
                for j in range(2):
                    for tl in range(4):
                        filler.append(
                            lambda j=j, tl=tl: v_quantum(xp4, 4, j, tl))
                filler.append(lambda: k_quantum(xp6, 6, 0))
                filler.append(lambda: k_quantum(xp6, 6, 1))
                for j in range(2):
                    for tl in range(4):
                        filler.append(
                            lambda j=j, tl=tl: v_quantum(xp6, 6, j, tl))
                for nt in range(4):
                    filler.append(lambda nt=nt: cp(nt))

                fill_pos = 0

                def pull(n):
                    nonlocal fill_pos
                    for _ in range(n):
                        if fill_pos < len(filler):
                            filler[fill_pos]()
                            fill_pos += 1

                def pull_through(idx):
                    nonlocal fill_pos
                    while fill_pos <= idx:
                        filler[fill_pos]()
                        fill_pos += 1

                # filler indices: 0-1 k0, 2-3 qp(qc0), 4-11 v(pair0),
                # 12-13 qp(qc1), 14-15 k2, 16-23 v2, 24-25 k4, 26-33 v4,
                # 34-35 k6, 36-43 v6, 44-45 cp(0..1)
                IDX_QP0 = 3
                IDX_K2 = 15
                IDX_K4 = 25
                IDX_K6 = 35

                # --- Unit 0 = (ot 0, qc 0): 16 diagonal tiles ---
                pull_through(IDX_QP0)          # k0 + q(qc0)
                for t in range(0, 8):
                    S(0, 0, t)
                    if t >= 2:
                        A(0, 0, t - 2)
                    pull(1)                    # v(pair0), q(qc1) stream in
                pull_through(IDX_K2)
                for t in range(8, 16):
                    S(0, 0, t)
                    A(0, 0, t - 2)
                    pull(1)
                A(0, 0, 14)
                A(0, 0, 15)
                norm_dve(0, 0)
                # --- Unit 1 = (1, 0): next unit's scores + filler cover the
                # norm chain and av-psum handover ---
                S(1, 0, 0)
                S(1, 0, 1)
                norm_fin(0, 0)
                pull(2)                        # k4 quanta
                for t in range(2, 16):
                    S(1, 0, t)
                    A(1, 0, t - 2)
                    if t in (4, 8, 12):
                        pull(1)
                A(1, 0, 14)
                A(1, 0, 15)
                norm_dve(1, 0)
                pull_through(IDX_K4)
                # --- Unit 2 = (0, 1): 32 tiles; also pre-emit the first 12 of
                # unit 3's full-tile scores so its exp runs ahead on ScalarE.
                # Filler is paced to last through the whole unit (ScalarE is
                # the laggard here; the PE needs projection work to chew on) ---
                S(0, 1, 0)
                S(0, 1, 1)
                pull(1)
                norm_fin(1, 0)
                u3_t = 0
                for t in range(2, 32):
                    S(0, 1, t)
                    A(0, 1, t - 2)
                    if t % 2 == 0 or t > 25:
                        pull(1)
                    if t % 2 == 1 and u3_t < 12:
                        S(1, 1, u3_t)
                        u3_t += 1
                    if t == 23:
                        pull_through(IDX_K6)
                A(0, 1, 30)
                A(0, 1, 31)
                norm_dve(0, 1)
                # --- Unit 3 = (1, 1): finish its full-tile scores (covers
                # unit 2's norm chain), then interleave full-tile av with
                # diagonal scores+av so the PE tracks ScalarE's exp progress.
                # c-proj kk0 matmuls for nt 4..6 pre-run here (they only need
                # unit 2's normalized half); kk1 + stores follow the last
                # norm, leaving a ~2us tail.
                pull(99)
                norm_fin(0, 1)
                while u3_t < 16:
                    S(1, 1, u3_t)
                    u3_t += 1
                S(1, 1, 16)
                S(1, 1, 17)
                po_pre = {}
                for i in range(14):
                    A(1, 1, i)            # full tile (start group at i == 0)
                    if i >= 3:
                        A(1, 1, 13 + i)   # diagonal tiles 16..26
                    if 18 + i < 32:
                        S(1, 1, 18 + i)
                # last scores are out; scp pool is safe to borrow for c-proj
                po_pre[4] = [pp.tile([128, 512], FP32, tag="proj",
                                     name=f"po4{ec}")[:] for ec in range(2)]
                cp_kk0(4, po_pre[4])
                A(1, 1, 14)
                A(1, 1, 27)
                sc5 = scp.tile([128, 1024], FP32, tag="sc", name="po5")
                po_pre[5] = [sc5[:, ts(ec, 512)] for ec in range(2)]
                cp_kk0(5, po_pre[5])
                A(1, 1, 15)
                A(1, 1, 28)
                sc6 = scp.tile([128, 1024], FP32, tag="sc", name="po6")
                po_pre[6] = [sc6[:, ts(ec, 512)] for ec in range(2)]
                cp_kk0(6, po_pre[6])
                for t in (29, 30, 31):
                    A(1, 1, t)            # stop group fires on tile 31
                norm_dve(1, 1)
                norm_fin(1, 1)
                for nt in range(4, 7):
                    cp_kk1(nt, po_pre[nt])
                cp(7)
            else:
                # unmasked fallback: coarse sequential schedule
                sxsl = [
                    xsl_pool.tile([128, NCC, 512], FP16, tag="xsl", name=f"sxsl{qc}")
                    for qc in range(NQC)
                ]
                for cc in range(NCC):
                    nc.sync.dma_start(wq_sb[:, cc, :], wq[ts(cc, 128), :])
                    nc.sync.dma_start(sxsl[0][:, cc, :], sxT[ts(cc, 128), ts(0, 512)])
                for cc in range(NCC):
                    nc.sync.dma_start(wk_sb[:, cc, :], wk[ts(cc, 128), :])
                    nc.sync.dma_start(sxsl[1][:, cc, :], sxT[ts(cc, 128), ts(1, 512)])
                nc.sync.dma_start(wv_sb[:], wv.rearrange("(cc p) o -> p cc o", p=128))
                nc.sync.dma_start(mask_sb[:], maskd[:])
                for kk in range(2):
                    nc.sync.dma_start(wc_sb[:, kk, :], wc[ts(kk, 128), :])
                for qc in range(NQC):
                    for ot in range(2):
                        qp_quantum(sxsl, qc, ot)
                for kc in range(0, NKC, 2):
                    xp = dma_pair_x(kc)
                    for ot in range(2):
                        k_quantum(xp, kc, ot)
                    for j in range(2):
                        for tl in range(4):
                            v_quantum(xp, kc, j, tl)
                for qc in range(NQC):
                    for ot in range(2):
                        for t in range(NTT):
                            S(ot, qc, t)
                            if t >= 2:
                                A(ot, qc, t - 2)
                        A(ot, qc, NTT - 2)
                        A(ot, qc, NTT - 1)
                        norm_dve(ot, qc)
                        norm_fin(ot, qc)
                for nt in range(NQT):
                    cp(nt)

    nc.compile()
    return nc


_NC_CACHE = {}


def _get_nc(masked: bool):
    if masked not in _NC_CACHE:
        _NC_CACHE[masked] = build_nc(masked)
    return _NC_CACHE[masked]


def _shard_inputs(x, sx, Wq, Wk, Wv, Wc, qm):
    f16 = np.float16
    bf = ml_dtypes.bfloat16
    t_len = x.shape[1]
    qscale = math.log(t_len) / math.sqrt(D)
    qmfull = np.tile(np.asarray(qm, np.float32), 4) * qscale  # (256,)

    tk = np.arange(128)[:, None]
    cl = np.arange(32)[None, :]
    mask = (cl >= tk // 4).astype(np.float32).astype(bf)

    in_maps = []
    for b in range(B):
        xT = np.ascontiguousarray(x[b].T).astype(f16)
        sxT = np.ascontiguousarray(sx[b].T).astype(f16)
        for hg in range(4):
            sl = slice(hg * HO, (hg + 1) * HO)
            in_maps.append(
                {
                    "xT": xT,
                    "sxT": sxT,
                    "wq": np.ascontiguousarray(
                        (Wq[sl, :] * qmfull[:, None]).T
                    ).astype(f16),
                    "wk": np.ascontiguousarray(Wk[sl, :].T).astype(f16),
                    "wv": np.ascontiguousarray(Wv[sl, :].T).astype(f16),
                    "wc": np.ascontiguousarray(Wc[:, sl].T).astype(f16),
                    "mask": mask,
                }
            )
    return in_maps


def _run(inputs, trace=False):
    from concourse.bass_utils import run_bass_kernel_spmd

    x = np.asarray(inputs["x"], np.float32)
    sx = np.asarray(inputs["sx"], np.float32)
    Wq = np.asarray(inputs["Wq"], np.float32)
    Wk = np.asarray(inputs["Wk"], np.float32)
    Wv = np.asarray(inputs["Wv"], np.float32)
    Wc = np.asarray(inputs["Wc"], np.float32)
    qm = np.asarray(inputs["qm"], np.float32)
    causal = int(np.asarray(inputs.get("causal", 1)))
    masked = bool(causal) and sx.shape[1] != x.shape[1]

    nc = _get_nc(masked)
    in_maps = _shard_inputs(x, sx, Wq, Wk, Wv, Wc, qm)
    kwargs = {}
    if trace:
        kwargs = dict(trace=True, trace_cores=list(range(8)))
    res = run_bass_kernel_spmd(nc, in_maps, core_ids=list(range(8)), **kwargs)

    out = np.zeros((B, ST, C), np.float32)
    for b in range(B):
        for hg in range(4):
            out[b] += np.asarray(res.results[b * 4 + hg]["out"], np.float32)
    return out, res


def kernel(**inputs):
    out, _ = _run(inputs, trace=False)
    return out


def kernel_traced(**inputs):
    out, res = _run(inputs, trace=True)
    return out, res


# revision 23
# speedup vs baseline: 1.3067x; 1.0121x over previous
"""Trainium2 Bass kernel for CombineAttention (B=2, T=4096, sT=1024, C=1024, H=16, D=64).

Sharding: 8 cores = 2 batches x 4 head-groups (4 heads each).
Host pre-transposes activations/weights so every on-device matmul has its
contraction dim on partitions; the monotonic mask (query i attends keys
<= 4i+3) becomes a block-causal structure handled by suffix-restricted
matmuls plus one small static (128,32) diagonal-band mask.

Precision: fp16 everywhere (full PE rate, FWL weight loads) except the
attention-weights path: exp(scores) can reach e^40, beyond fp16 range,
so exp and v are bf16 and the attn@v matmul runs in bf16. PSUM
accumulation is fp32; softmax needs no max-subtraction, and a
ones-column appended to v yields the softmax normalizer for free.
Output partials are stored fp16 and summed on host in fp32.

Schedule notes (what the ~650ns/descriptor DMA issue cost, the HAM clock
gate, and ScalarE exp throughput force):
  - DMA descriptors are big (128-256KB) and split across the sync /
    scalar / gpsimd queues so issue runs in parallel;
  - q/k/v projections are chopped into 1-3.5us quanta interleaved
    between attention tiles so the PE tracks ScalarE's exp rate and the
    HAM clock gate never sees an idle window;
  - the last unit's full-tile scores are emitted early (deep ex buffer)
    because it has no projection work left to hide exp latency;
  - v's stationary is padded to 128 columns so FWL hides the per-matmul
    weight load in the attn@v stream;
  - narrow diagonal-tile pairs share one ACTIVATE (fewer 352-cycle
    fixed costs on the ScalarE critical path);
  - softmax normalization broadcasts 1/l via a K=1 PE matmul; the final
    unit's chain is split across Vector/Scalar/GpSimd engines;
  - c-projection runs mid-kernel for query chunk 0, and its kk0 half
    pre-runs for chunk 1, leaving only 10 matmuls after the last norm.
"""

import math
from contextlib import ExitStack

import numpy as np
import ml_dtypes

import concourse.bass as bass
import concourse.tile as tile
from concourse import bacc, mybir
from concourse.bass import ts, ds

BF16 = mybir.dt.bfloat16
FP16 = mybir.dt.float16
FP32 = mybir.dt.float32

B = 2
C = 1024
T = 4096
ST = 1024
H = 16
D = 64
HO = 256          # head-group output channels per core (4 heads)
NCC = C // 128    # 8 contraction chunks
NTT = T // 128    # 32 key tiles
NKC = T // 512    # 8 key slices (projection streaming)
NQC = ST // 512   # 2 query chunks (attention)
NQT = ST // 128   # 8 query tiles (c-projection)
WARM_MMS = 13     # PE warmup burst bridging the DMA prologue
FAST_TAIL = False   # split final norm/store chains across Vector/Scalar/GpSimd
STORE_SPLIT = False  # out stores alternate sync/gpsimd DMA queues
PACK_ACT = False    # pack two narrow diagonal tiles into one ACTIVATE


def build_nc(masked: bool = True):
    nc = bacc.Bacc("TRN2", target_bir_lowering=False, debug=False, num_devices=8)
    xT = nc.dram_tensor("xT", [C, T], FP16, kind="ExternalInput").ap()
    sxT = nc.dram_tensor("sxT", [C, ST], FP16, kind="ExternalInput").ap()
    wq = nc.dram_tensor("wq", [C, HO], FP16, kind="ExternalInput").ap()
    wk = nc.dram_tensor("wk", [C, HO], FP16, kind="ExternalInput").ap()
    wv = nc.dram_tensor("wv", [C, HO], FP16, kind="ExternalInput").ap()
    wc = nc.dram_tensor("wc", [HO, C], FP16, kind="ExternalInput").ap()
    maskd = nc.dram_tensor("mask", [128, 32], BF16, kind="ExternalInput").ap()
    out = nc.dram_tensor("out", [ST, C], FP16, kind="ExternalOutput").ap()

    with tile.TileContext(nc) as tc, ExitStack() as ctx:
        const = ctx.enter_context(tc.tile_pool(name="const", bufs=1))
        big = ctx.enter_context(tc.tile_pool(name="big", bufs=1))
        xsl_pool = ctx.enter_context(tc.tile_pool(name="xsl", bufs=6))
        work = ctx.enter_context(tc.tile_pool(name="work", bufs=20))
        nrm = ctx.enter_context(tc.tile_pool(name="nrm", bufs=4))
        outw = ctx.enter_context(tc.tile_pool(name="outw", bufs=4))

        wq_sb = const.tile([128, NCC, HO], FP16, tag="wq")
        wk_sb = const.tile([128, NCC, HO], FP16, tag="wk")
        wv_sb = const.tile([128, NCC, HO], FP16, tag="wv")
        wc_sb = const.tile([128, 2, C], FP16, tag="wc")
        mask_sb = const.tile([128, 32], BF16, tag="mask")
        warm_sb = const.tile([128, 512], BF16, tag="warm")
        ones_sb = const.tile([1, 64], BF16, tag="ones")

        kT_sb = big.tile([128, 2, T], FP16, tag="kT")
        qsT_sb = big.tile([128, 2, ST], FP16, tag="qsT")
        # v stationary padded to 128 columns: col 64 = softmax-normalizer
        # ones, cols 65..127 = zeros so FWL (needs exactly 128 weight
        # columns) hides the per-matmul weight load in the attn@v stream
        v_sb = big.tile([128, NTT, 4, 128], BF16, tag="v")
        yT_sb = [
            big.tile([128, 2, 512], FP16, tag=f"yT{qc}", name=f"yT{qc}")
            for qc in range(NQC)
        ]

        nc.gpsimd.memset(warm_sb[:], 0.125)
        nc.gpsimd.memset(ones_sb[:], 1.0)
        nc.vector.memset(v_sb[:, :, :, 64:65], 1.0)
        nc.vector.memset(v_sb[:, :, :, 65:128], 0.0)

        with tc.tile_pool(name="psA", bufs=2, space="PSUM") as pp, \
             tc.tile_pool(name="psS", bufs=2, space="PSUM") as scp, \
             tc.tile_pool(name="psV", bufs=2, space="PSUM") as avp:

            # ---- PE warmup: bridge from kernel start until the first
            # x/weight slices land (the HAM clock gate needs ~3.4us of
            # sustained matmul activity to lift the 1.2GHz throttle) ----
            wps = pp.tile([128, 512], FP32, tag="proj", name="warmps")
            for i in range(WARM_MMS):
                nc.tensor.matmul(
                    wps[:], warm_sb[:, 0:128], warm_sb[:], start=True, stop=True
                )

            # ---------------- DMA emission helpers ----------------
            # Each dma_start costs ~650ns of ISSUE time on its engine's
            # sequencer (serial, in-order), and each descriptor lands on one
            # HW queue (~22GB/s). Descriptors are kept big and spread across
            # the sync/scalar/gpsimd queues so issue runs in parallel.
            def dma_pair_x(kc0, fine=False):
                """Start DMAs for key slices kc0, kc0+1; returns xsl tiles.
                fine=True: per-cc 128KB descriptors in consumption order
                (lower first-chunk latency); else 2-cc 256KB descriptors."""
                xsl = [
                    xsl_pool.tile(
                        [128, NCC, 512], FP16, tag="xsl", name=f"xsl{kc0 + j}"
                    )
                    for j in range(2)
                ]
                if fine:
                    for cc in range(NCC):
                        for j in range(2):
                            nc.sync.dma_start(
                                xsl[j][:, cc, :],
                                xT[ts(cc, 128), ts(kc0 + j, 512)])
                else:
                    for j in range(2):
                        for cc0 in range(0, NCC, 2):
                            nc.sync.dma_start(
                                xsl[j][:, cc0:cc0 + 2, :],
                                xT[ds(cc0 * 128, 256),
                                   ts(kc0 + j, 512)].rearrange(
                                    "(c p) w -> p c w", p=128),
                            )
                return xsl

            # ---------------- PE work quanta ----------------
            def qp_quantum(sxsl, qc, ot):
                """q projection for (query chunk qc, channel half ot): 8 MMs."""
                ps = pp.tile([128, 512], FP32, tag="proj", name=f"pq{qc}{ot}")
                for cc in range(NCC):
                    nc.tensor.matmul(
                        ps[:],
                        wq_sb[:, cc, ts(ot, 128)],
                        sxsl[qc][:, cc, :],
                        start=(cc == 0),
                        stop=(cc == NCC - 1),
                    )
                nc.vector.tensor_copy(qsT_sb[:, ot, ts(qc, 512)], ps[:])

            def k_quantum(xsl, kc0, ot):
                """k projection for slices kc0,kc0+1 (one channel half): 16 MMs
                sharing stationary loads across the j-pair."""
                pk = [pp.tile([128, 512], FP32, tag="proj", name=f"pk{j}")
                      for j in range(2)]
                for cc in range(NCC):
                    for j in range(2):
                        nc.tensor.matmul(
                            pk[j][:],
                            wk_sb[:, cc, ts(ot, 128)],
                            xsl[j][:, cc, :],
                            start=(cc == 0),
                            stop=(cc == NCC - 1),
                        )
                for j in range(2):
                    nc.vector.tensor_copy(kT_sb[:, ot, ts(kc0 + j, 512)], pk[j][:])

            def v_quantum(xsl, kc0, j, tl):
                """v projection for one 128-key tile: 8 MMs of N=256."""
                tt = 4 * (kc0 + j) + tl
                ps = pp.tile([128, 512], FP32, tag="proj", name="pv")
                pv = ps[:, 0:256]
                for cc in range(NCC):
                    nc.tensor.matmul(
                        pv,
                        xsl[j][:, cc, ts(tl, 128)],
                        wv_sb[:, cc, :],
                        start=(cc == 0),
                        stop=(cc == NCC - 1),
                    )
                nc.vector.tensor_copy(
                    v_sb[:, tt, :, 0:64], pv.rearrange("p (h d) -> p h d", h=4)
                )

            ex_tiles = {}
            av_tiles = {}

            def tile_geom(qc, tt):
                r = tt - 16 * qc if masked else -1  # >= 0: diagonal-band tile
                col0 = 32 * r if r >= 0 else 0
                width = 512 - col0
                return r, col0, width

            def ntiles_of(qc):
                return (16 * (qc + 1)) if masked else NTT

            def _score_mms(ot, qc, tt, sc, off, width, col0):
                # the two heads' matmuls run concurrently on the upper/lower
                # halves of the PE array (row tiling via base_partition)
                for h in range(2):
                    row = ds(64 * h, 64)
                    nc.tensor.matmul(
                        sc[:, ds(off + width * h, width)],
                        kT_sb[row, ot, ts(tt, 128)],
                        qsT_sb[row, ot, ds(512 * qc + col0, width)],
                        start=True,
                        stop=True,
                    )

            def _mask_mul(ex, off, width):
                exb = ex[:, ds(off, 2 * width)].rearrange(
                    "p (g x) -> p g x", g=2
                )[:, :, 0:32]
                nc.vector.tensor_mul(
                    exb, exb, mask_sb[:].unsqueeze(1).broadcast_to([128, 2, 32])
                )

            def S(ot, qc, tt):
                """scoresT + exp + band-mask for one 128-key tile of heads
                (2*ot, 2*ot+1), queries [512*qc, 512*qc+512)."""
                r, col0, width, = tile_geom(qc, tt)
                base = 512 - width
                # both heads' scores go into one 2-bank psum tile, h0 at the
                # end of bank 0 and h1 at the start of bank 1, so a single
                # gap-free ACTIVATE (352-cycle fixed cost) covers the pair
                sc = scp.tile([128, 1024], FP32, tag="sc")
                _score_mms(ot, qc, tt, sc, base, width, col0)
                ex = work.tile([128, 1024], BF16, tag="exp", name=f"ex{ot}{qc}{tt}")
                nc.scalar.activation(
                    ex[:, ds(base, 2 * width)],
                    sc[:, ds(base, 2 * width)],
                    mybir.ActivationFunctionType.Exp,
                )
                if r >= 0:
                    _mask_mul(ex, base, width)
                ex_tiles[(ot, qc, tt)] = (ex, base, width)

            def S2(ot, qc, t1):
                """two narrow diagonal tiles (width <= 256) share one psum
                tile and one ACTIVATE, halving the 352-cycle fixed cost. No
                matmul output segment may cross the psum bank boundary, so
                tile t1+1 starts at column 512 unless both pairs fit bank 0."""
                if not PACK_ACT:
                    S(ot, qc, t1)
                    S(ot, qc, t1 + 1)
                    return
                _, col0a, wa = tile_geom(qc, t1)
                _, col0b, wb = tile_geom(qc, t1 + 1)
                offb = 2 * wa if 2 * (wa + wb) <= 512 else 512
                sc = scp.tile([128, 1024], FP32, tag="sc")
                _score_mms(ot, qc, t1, sc, 0, wa, col0a)
                _score_mms(ot, qc, t1 + 1, sc, offb, wb, col0b)
                ex = work.tile([128, 1024], BF16, tag="exp", name=f"ex{ot}{qc}{t1}p")
                nc.scalar.activation(
                    ex[:, 0:offb + 2 * wb],
                    sc[:, 0:offb + 2 * wb],
                    mybir.ActivationFunctionType.Exp,
                )
                _mask_mul(ex, 0, wa)
                _mask_mul(ex, offb, wb)
                ex_tiles[(ot, qc, t1)] = (ex, 0, wa)
                ex_tiles[(ot, qc, t1 + 1)] = (ex, offb, wb)

            def A(ot, qc, tt):
                """attn @ v_aug accumulation for one key tile."""
                ntiles = ntiles_of(qc)
                if tt == 0:
                    av_tiles[(ot, qc)] = [
                        avp.tile([128, 512], FP32, tag="av", name=f"av{ot}{qc}{hh}")
                        for hh in range(2)
                    ]
                avps = av_tiles[(ot, qc)]
                _, col0, width = tile_geom(qc, tt)
                ex, off, _w = ex_tiles.pop((ot, qc, tt))
                for h in range(2):
                    nc.tensor.matmul(
                        avps[h][:, ds(col0, width)],
                        v_sb[:, tt, 2 * ot + h, :],
                        ex[:, ds(off + width * h, width)],
                        start=(tt == 0),
                        stop=(tt == ntiles - 1),
                    )

            norm_state = {}

            def norm_dve(ot, qc, fast=False):
                """First half of y = yT_unnorm / l: copy av+l to SBUF and
                compute 1/l. fast=True splits the chains across Vector /
                Scalar / GpSimd (only safe when ScalarE has no exp backlog)."""
                avps = av_tiles.pop((ot, qc))
                st = []
                for h in range(2):
                    lsb = nrm.tile([1, 512], FP32, tag="lsb")
                    if fast and h == 1 and FAST_TAIL:
                        nc.scalar.copy(lsb[:], avps[h][64:65, :])
                    else:
                        nc.vector.tensor_copy(lsb[:], avps[h][64:65, :])
                    linv = nrm.tile([1, 512], FP32, tag="linv")
                    # custom-DVE recip needs a partition-0 SBUF input
                    nc.vector.reciprocal_approx_fast(linv[:], lsb[:])
                    linvb = nrm.tile([1, 512], BF16, tag="linvb")
                    if fast and FAST_TAIL:
                        nc.gpsimd.tensor_copy(linvb[:], linv[:])
                    else:
                        nc.vector.tensor_copy(linvb[:], linv[:])
                    st.append([None, linvb])
                for h in range(2):
                    avsb = nrm.tile([64, 512], FP32, tag="avsb", name=f"avsb{h}")
                    if fast and h == 1 and FAST_TAIL:
                        nc.scalar.copy(avsb[:], avps[h][0:64, :])
                    else:
                        nc.vector.tensor_copy(avsb[:], avps[h][0:64, :])
                    st[h][0] = avsb
                norm_state[(ot, qc)] = st

            def norm_fin(ot, qc):
                """Second half: K=1 PE matmul broadcasts 1/l across 64
                partitions, one DVE multiply writes normalized yT (fp16).
                Must precede the next unit's first av matmul (psum reuse)."""
                st = norm_state.pop((ot, qc))
                bcs = []
                for h, (avsb, linvb) in enumerate(st):
                    bc = avp.tile([64, 512], FP32, tag="av", name=f"bc{h}")
                    nc.tensor.matmul(
                        bc[:], ones_sb[:], linvb[:], start=True, stop=True
                    )
                    bcs.append(bc)
                for h, (avsb, linvb) in enumerate(st):
                    nc.vector.tensor_mul(
                        yT_sb[qc][ds(64 * h, 64), ot, :], avsb[:],
                        bcs[h][:]
                    )

            def _store(nt, ec, po_ap, fast, nsplit=1):
                osb = outw.tile([128, 512], FP16, tag="osb")
                if fast and ec == 1 and FAST_TAIL:
                    nc.scalar.copy(osb[:], po_ap)
                else:
                    nc.vector.tensor_copy(osb[:], po_ap)
                if not STORE_SPLIT:
                    nsplit = 1
                w = 512 // nsplit
                for sp in range(nsplit):
                    if STORE_SPLIT and (ec + sp) % 2 == 0:
                        eng = nc.sync
                    else:
                        eng = nc.gpsimd
                    eng.dma_start(
                        out[ts(nt, 128), ds(ec * 512 + sp * w, w)],
                        osb[:, ds(sp * w, w)])

            def cp(nt, fast=False, nsplit=1):
                """c-projection + fp16 store for one 128-query tile."""
                po = [pp.tile([128, 512], FP32, tag="proj", name=f"po{ec}")
                      for ec in range(2)]
                for kk in range(2):
                    for ec in range(2):
                        nc.tensor.matmul(
                            po[ec][:],
                            yT_sb[nt // 4][:, kk, ts(nt % 4, 128)],
                            wc_sb[:, kk, ts(ec, 512)],
                            start=(kk == 0),
                            stop=(kk == 1),
                        )
                for ec in range(2):
                    _store(nt, ec, po[ec][:], fast, nsplit)

            def cp_kk0(nt, po):
                """first-half (kk=0) c-proj matmuls into caller-provided
                psum APs; pre-runnable once unit (0, qc) is normalized."""
                for ec in range(2):
                    nc.tensor.matmul(
                        po[ec],
                        yT_sb[nt // 4][:, 0, ts(nt % 4, 128)],
                        wc_sb[:, 0, ts(ec, 512)],
                        start=True,
                        stop=False,
                    )

            def cp_kk1(nt, po, fast=False, nsplit=1):
                """second-half (kk=1) c-proj matmuls + fp16 store."""
                for ec in range(2):
                    nc.tensor.matmul(
                        po[ec],
                        yT_sb[nt // 4][:, 1, ts(nt % 4, 128)],
                        wc_sb[:, 1, ts(ec, 512)],
                        start=False,
                        stop=True,
                    )
                for ec in range(2):
                    _store(nt, ec, po[ec], fast, nsplit)

            # ---------------- masked (monotonic) schedule ----------------
            if masked:
                sxsl = [
                    xsl_pool.tile([128, NCC, 512], FP16, tag="xsl", name=f"sxsl{qc}")
                    for qc in range(NQC)
                ]
                # Issue plan: sync carries the x pairs (the long pole);
                # gpsimd carries wk/wv/mask (first k/v quanta); scalar
                # carries wq/sx/wc (q projection). All three issue in
                # parallel from kernel start.
                xp0 = dma_pair_x(0, fine=True)
                for cc in range(NCC):
                    nc.gpsimd.dma_start(wk_sb[:, cc, :], wk[ts(cc, 128), :])
                for cc0 in range(0, NCC, 2):
                    nc.scalar.dma_start(
                        wq_sb[:, cc0:cc0 + 2, :],
                        wq[ds(cc0 * 128, 256), :].rearrange(
                            "(c p) o -> p c o", p=128))
                    nc.scalar.dma_start(
                        sxsl[0][:, cc0:cc0 + 2, :],
                        sxT[ds(cc0 * 128, 256), ts(0, 512)].rearrange(
                            "(c p) w -> p c w", p=128))
                for cc0 in range(0, NCC, 2):
                    nc.gpsimd.dma_start(
                        wv_sb[:, cc0:cc0 + 2, :],
                        wv[ds(cc0 * 128, 256), :].rearrange(
                            "(c p) o -> p c o", p=128))
                nc.gpsimd.dma_start(mask_sb[:], maskd[:])
                for cc0 in range(0, NCC, 2):
                    nc.scalar.dma_start(
                        sxsl[1][:, cc0:cc0 + 2, :],
                        sxT[ds(cc0 * 128, 256), ts(1, 512)].rearrange(
                            "(c p) w -> p c w", p=128))
                for kk in range(2):
                    nc.scalar.dma_start(wc_sb[:, kk, :], wc[ts(kk, 128), :])
                xp2 = dma_pair_x(2)
                # pair4 reuses sxsl0's buffer (waits on q-proj qc0), pair6
                # reuses pair0's (waits on k/v of pair0) — emitted now, the
                # semaphores resolve the timing.
                xp4 = dma_pair_x(4)
                xp6 = dma_pair_x(6)

                # Filler queue: projection/c-proj quanta consumed between
                # attention tiles. Order respects data availability.
                filler = []
                filler.append(lambda: k_quantum(xp0, 0, 0))
                filler.append(lambda: k_quantum(xp0, 0, 1))
                filler.append(lambda: qp_quantum(sxsl, 0, 0))
                filler.append(lambda: qp_quantum(sxsl, 0, 1))
                for j in range(2):
                    for tl in range(4):
                        filler.append(
                            lambda j=j, tl=tl: v_quantum(xp0, 0, j, tl))
                filler.append(lambda: qp_quantum(sxsl, 1, 0))
                filler.append(lambda: qp_quantum(sxsl, 1, 1))
                filler.append(lambda: k_quantum(xp2, 2, 0))
                filler.append(lambda: k_quantum(xp2, 2, 1))
                for j in range(2):
                    for tl in range(4):
                        filler.append(
                            lambda j=j, tl=tl: v_quantum(xp2, 2, j, tl))
                filler.append(lambda: k_quantum(xp4, 4, 0))
                filler.append(lambda: k_quantum(xp4, 4, 1))
                for j in range(2):
                    for tl in range(4):
                        filler.append(
                            lambda j=j, tl=tl: v_quantum(xp4, 4, j, tl))
                filler.append(lambda: k_quantum(xp6, 6, 0))
                filler.append(lambda: k_quantum(xp6, 6, 1))
                for j in range(2):
                    for tl in range(4):
                        filler.append(
                            lambda j=j, tl=tl: v_quantum(xp6, 6, j, tl))
                for nt in range(4):
                    filler.append(lambda nt=nt: cp(nt))

                fill_pos = 0

                def pull(n):
                    nonlocal fill_pos
                    for _ in range(n):
                        if fill_pos < len(filler):
                            filler[fill_pos]()
                            fill_pos += 1

                def pull_through(idx):
                    nonlocal fill_pos
                    while fill_pos <= idx:
                        filler[fill_pos]()
                        fill_pos += 1

                # filler indices: 0-1 k0, 2-3 qp(qc0), 4-11 v(pair0),
                # 12-13 qp(qc1), 14-15 k2, 16-23 v2, 24-25 k4, 26-33 v4,
                # 34-35 k6, 36-43 v6, 44-47 cp(0..3)
                IDX_QP0 = 3
                IDX_K2 = 15

                # --- Unit 0 = (ot 0, qc 0): 16 diagonal tiles ---
                pull_through(IDX_QP0)          # k0 + q(qc0)
                for t in range(0, 8):
                    S(0, 0, t)
                    if t >= 2:
                        A(0, 0, t - 2)
                    pull(1)                    # v(pair0) streams in
                pull_through(IDX_K2)           # q(qc1) + k2
                for t in (8, 10, 12, 14):
                    S2(0, 0, t)
                    A(0, 0, t - 2)
                    A(0, 0, t - 1)
                    pull(2)                    # v(pair2)
                A(0, 0, 14)
                A(0, 0, 15)
                norm_dve(0, 0)
                # --- Unit 1 = (1, 0): next unit's scores + k4 cover the
                # norm chain and av-psum handover ---
                S(1, 0, 0)
                S(1, 0, 1)
                norm_fin(0, 0)
                pull(2)                        # k4
                for t in range(2, 8):
                    S(1, 0, t)
                    A(1, 0, t - 2)
                    if t == 4:
                        pull(1)                # v4 #1
                for t in (8, 10, 12, 14):
                    S2(1, 0, t)
                    A(1, 0, t - 2)
                    A(1, 0, t - 1)
                A(1, 0, 14)
                A(1, 0, 15)
                norm_dve(1, 0)
                # --- Unit 2 = (0, 1): 32 tiles; also pre-emit the first 10
                # of unit 3's full-tile scores so its exp runs ahead on
                # ScalarE. Filler paced to last the whole unit (ScalarE is
                # the laggard here) ---
                S(0, 1, 0)
                S(0, 1, 1)
                pull(1)                        # v4 #2
                norm_fin(1, 0)
                u3_t = 0
                for t in range(2, 24):
                    S(0, 1, t)
                    A(0, 1, t - 2)
                    if t % 2 == 0:
                        pull(1)
                    elif u3_t < 10:
                        S(1, 1, u3_t)
                        u3_t += 1
                for t in (24, 26, 28, 30):
                    S2(0, 1, t)
                    A(0, 1, t - 2)
                    A(0, 1, t - 1)
                    pull(2)
                A(0, 1, 30)
                A(0, 1, 31)
                pull(1)                        # cp(3)
                norm_dve(0, 1)
                # --- Unit 3 = (1, 1): finish its full-tile scores (covers
                # unit 2's norm chain), then interleave full-tile av with
                # diagonal scores+av so the PE tracks ScalarE's exp progress;
                # c-proj kk0 for nt 4..6 pre-runs near the end ---
                while u3_t < 16:
                    S(1, 1, u3_t)
                    u3_t += 1
                norm_fin(0, 1)
                S(1, 1, 16)
                S(1, 1, 17)
                po_pre = {}
                for i in range(16):
                    A(1, 1, i)            # full tile (start group at i == 0)
                    if i >= 3:
                        A(1, 1, 13 + i)   # diagonal tiles 16..28
                    if i < 6:
                        S(1, 1, 18 + i)   # diagonal 18..23 individually
                    elif i in (6, 8, 10, 12):
                        S2(1, 1, 18 + i)  # diagonal pairs (24,25)..(30,31)
                    if i == 13:
                        po_pre[4] = [pp.tile([128, 512], FP32, tag="proj",
                                             name=f"po4{ec}")[:]
                                     for ec in range(2)]
                        cp_kk0(4, po_pre[4])
                    elif i == 14:
                        sc5 = scp.tile([128, 1024], FP32, tag="sc", name="po5")
                        po_pre[5] = [sc5[:, ts(ec, 512)] for ec in range(2)]
                        cp_kk0(5, po_pre[5])
                    elif i == 15:
                        sc6 = scp.tile([128, 1024], FP32, tag="sc", name="po6")
                        po_pre[6] = [sc6[:, ts(ec, 512)] for ec in range(2)]
                        cp_kk0(6, po_pre[6])
                for t in (29, 30, 31):
                    A(1, 1, t)            # stop group fires on tile 31
                norm_dve(1, 1, fast=True)
                norm_fin(1, 1)
                for nt in range(4, 7):
                    cp_kk1(nt, po_pre[nt], fast=True)
                cp(7, fast=True, nsplit=2)
            else:
                # unmasked fallback: coarse sequential schedule
                sxsl = [
                    xsl_pool.tile([128, NCC, 512], FP16, tag="xsl", name=f"sxsl{qc}")
                    for qc in range(NQC)
                ]
                for cc in range(NCC):
                    nc.sync.dma_start(wq_sb[:, cc, :], wq[ts(cc, 128), :])
                    nc.sync.dma_start(sxsl[0][:, cc, :], sxT[ts(cc, 128), ts(0, 512)])
                for cc in range(NCC):
                    nc.sync.dma_start(wk_sb[:, cc, :], wk[ts(cc, 128), :])
                    nc.sync.dma_start(sxsl[1][:, cc, :], sxT[ts(cc, 128), ts(1, 512)])
                for cc in range(NCC):
                    nc.gpsimd.dma_start(wv_sb[:, cc, :], wv[ts(cc, 128), :])
                nc.gpsimd.dma_start(mask_sb[:], maskd[:])
                for kk in range(2):
                    nc.sync.dma_start(wc_sb[:, kk, :], wc[ts(kk, 128), :])
                for qc in range(NQC):
                    for ot in range(2):
                        qp_quantum(sxsl, qc, ot)
                for kc in range(0, NKC, 2):
                    xp = dma_pair_x(kc)
                    for ot in range(2):
                        k_quantum(xp, kc, ot)
                    for j in range(2):
                        for tl in range(4):
                            v_quantum(xp, kc, j, tl)
                for qc in range(NQC):
                    for ot in range(2):
                        for t in range(NTT):
                            S(ot, qc, t)
                            if t >= 2:
                                A(ot, qc, t - 2)
                        A(ot, qc, NTT - 2)
                        A(ot, qc, NTT - 1)
                        norm_dve(ot, qc)
                        norm_fin(ot, qc)
                for nt in range(NQT):
                    cp(nt)

    nc.compile()
    return nc


_NC_CACHE = {}


def _get_nc(masked: bool):
    if masked not in _NC_CACHE:
        _NC_CACHE[masked] = build_nc(masked)
    return _NC_CACHE[masked]


def _shard_inputs(x, sx, Wq, Wk, Wv, Wc, qm):
    f16 = np.float16
    bf = ml_dtypes.bfloat16
    t_len = x.shape[1]
    qscale = math.log(t_len) / math.sqrt(D)
    qmfull = np.tile(np.asarray(qm, np.float32), 4) * qscale  # (256,)

    tk = np.arange(128)[:, None]
    cl = np.arange(32)[None, :]
    mask = (cl >= tk // 4).astype(np.float32).astype(bf)

    in_maps = []
    for b in range(B):
        xT = np.ascontiguousarray(x[b].T).astype(f16)
        sxT = np.ascontiguousarray(sx[b].T).astype(f16)
        for hg in range(4):
            sl = slice(hg * HO, (hg + 1) * HO)
            in_maps.append(
                {
                    "xT": xT,
                    "sxT": sxT,
                    "wq": np.ascontiguousarray(
                        (Wq[sl, :] * qmfull[:, None]).T
                    ).astype(f16),
                    "wk": np.ascontiguousarray(Wk[sl, :].T).astype(f16),
                    "wv": np.ascontiguousarray(Wv[sl, :].T).astype(f16),
                    "wc": np.ascontiguousarray(Wc[:, sl].T).astype(f16),
                    "mask": mask,
                }
            )
    return in_maps


def _run(inputs, trace=False):
    from concourse.bass_utils import run_bass_kernel_spmd

    x = np.asarray(inputs["x"], np.float32)
    sx = np.asarray(inputs["sx"], np.float32)
    Wq = np.asarray(inputs["Wq"], np.float32)
    Wk = np.asarray(inputs["Wk"], np.float32)
    Wv = np.asarray(inputs["Wv"], np.float32)
    Wc = np.asarray(inputs["Wc"], np.float32)
    qm = np.asarray(inputs["qm"], np.float32)
    causal = int(np.asarray(inputs.get("causal", 1)))
    masked = bool(causal) and sx.shape[1] != x.shape[1]

    nc = _get_nc(masked)
    in_maps = _shard_inputs(x, sx, Wq, Wk, Wv, Wc, qm)
    kwargs = {}
    if trace:
        kwargs = dict(trace=True, trace_cores=list(range(8)))
    res = run_bass_kernel_spmd(nc, in_maps, core_ids=list(range(8)), **kwargs)

    out = np.zeros((B, ST, C), np.float32)
    for b in range(B):
        for hg in range(4):
            out[b] += np.asarray(res.results[b * 4 + hg]["out"], np.float32)
    return out, res


def kernel(**inputs):
    out, _ = _run(inputs, trace=False)
    return out


def kernel_traced(**inputs):
    out, res = _run(inputs, trace=True)
    return out, res
